# revision 23
# baseline (speedup 1.0000x reference)
"""Tensor-parallel causal attention block for Trainium2 (8 NeuronCores).

Sharding: tensor-parallel across heads for QKV+attention (2 heads/core),
then an AllToAll (fp16 payload, 4MB/core) to switch to row-parallel for
the output projection — much cheaper than the AllReduce the module's
TPLinear layout implies (64MB/core).

Dataflow per core: x^T is pre-transposed on the host so the C-contraction
sits on the partition axis. Q,K are produced transposed [d,t] with RoPE
fused into the PSUM eviction (cos/sin tables host-precomputed); V in
natural [t,d] layout. Scores are computed transposed (S^T = K·Q) so the
PV matmul needs no on-chip transposes at all. Softmax skips the
max-subtraction (scores are O(5) here, exp is fp32-safe), masks causality
with a host-built 0/1 tile (one sliced view per diagonal offset), gets the
denominator from a ones-vector matmul accumulated alongside PV, and folds
normalization into the PSUM eviction via reciprocal + a rank-1 broadcast
matmul.

All matmul inputs are fp16 (inputs cast on host, intermediates written as
fp16 by the evicting engine); accumulation stays fp32 in PSUM. Score
tiles are paired two-per-PSUM-allocation so one exp covers 1024 columns,
halving the S->exp->PV cross-engine sync hops.
"""
import numpy as np

import concourse.bass as bass
import concourse.tile as tile
import concourse.mybir as mybir
from concourse.bass_utils import run_bass_kernel_spmd

N_CORES = 8
B, T, C = 4, 2048, 2048
H = 16                 # total heads
HPC = H // N_CORES     # heads per core = 2
D = C // H             # head dim = 128
P = 128                # partitions
TG = 512               # t-group (moving free dim)
NTG = T // TG          # 4 groups per batch
NCC = C // P           # 16 contraction chunks
NSLICE = B * T // N_CORES  # 1024 output rows per core

FP = mybir.dt.float32
FPR = mybir.dt.float32r
FP16 = mybir.dt.float16
EXP = mybir.ActivationFunctionType.Exp
SCALE = 1.0 / float(np.sqrt(D))

# ---------------------------------------------------------------------------
# Workaround: this container's walrus rejects >1 sync-wait per instruction.
# Hoist extras onto preceding same-engine NoOps (engine streams are in-order).
# ---------------------------------------------------------------------------
from concourse.vector_clock import ScopedClock


def _fixup_multiwaits(nc):
    moved = 0
    for fn in nc.m.functions:
        for bb in fn.blocks:
            insts = bb.instructions
            if not any(
                i.sync_info and i.sync_info.on_wait and len(i.sync_info.on_wait) > 1
                for i in insts
            ):
                continue
            new_insts = []
            for ins in insts:
                si = ins.sync_info
                if si is not None and si.on_wait and len(si.on_wait) > 1:
                    extra, keep = si.on_wait[:-1], si.on_wait[-1:]
                    for w in extra:
                        nop = mybir.InstNoOp(
                            name=nc.get_next_instruction_name(),
                            ins=[],
                            outs=[],
                            engine=ins.engine,
                        )
                        nop.sync_info = mybir.SyncInfo(on_wait=[w], on_update=[])
                        new_insts.append(nop)
                        moved += 1
                    si.on_wait = keep
                new_insts.append(ins)
            bb.instructions = new_insts
    return moved


def _patched_drain_and_barrier(self, tick_clock, wait_clock):
    nop = self.nc.sync.nop(nofuse=True)
    wait_clock.add_sem_waits(nop.ins, ScopedClock({None: tick_clock.global_clock}))
    w = nop.ins.sync_info.on_wait if nop.ins.sync_info else []
    while w and len(w) > 1:
        cond = w.pop()
        n2 = self.nc.sync.nop(nofuse=True)
        if n2.ins.sync_info is None:
            n2.ins.sync_info = mybir.SyncInfo(on_wait=[], on_update=[])
        n2.ins.sync_info.on_wait.append(cond)
    self.nc.sync.drain()
    self.nc.all_engine_barrier()
    assert self.sems is not None
    popped = self.nc._tile_sem_poison_stack.pop()
    assert popped is self._sem_poison
    self.nc.clear_and_free_semaphores(list(self.sems.allocated().values()))
    self.nc.all_engine_barrier()


tile.TileContext._drain_and_barrier = _patched_drain_and_barrier

# SBUF cap: tile_utils caps at 192KB/partition; cayman has 208 usable.
try:
    import concourse.tile_utils as _tile_utils

    if getattr(_tile_utils, "max_sbuf_usage", None) is not None:
        _tile_utils.max_sbuf_usage = 204 * 1024
except Exception:
    pass


# ---------------------------------------------------------------------------
# Device program
# ---------------------------------------------------------------------------
def build_program(reps: int = 1, mode: str = "full"):
    nc = bass.Bass()

    xT = nc.dram_tensor("xT", [B, C, T], FP16, kind="ExternalInput")
    wqT = nc.dram_tensor("wqT", [C, HPC * D], FP16, kind="ExternalInput")
    wkT = nc.dram_tensor("wkT", [C, HPC * D], FP16, kind="ExternalInput")
    wvT = nc.dram_tensor("wvT", [C, HPC * D], FP16, kind="ExternalInput")
    woT = nc.dram_tensor("woT", [C, C], FP16, kind="ExternalInput")
    cos_t = nc.dram_tensor("cos_t", [D // 2, T], FP, kind="ExternalInput")
    sin_t = nc.dram_tensor("sin_t", [D // 2, T], FP, kind="ExternalInput")
    maskc = nc.dram_tensor("maskc", [P, 896], FP16, kind="ExternalInput")
    ones_col = nc.dram_tensor("ones_col", [P, 1], FP16, kind="ExternalInput")
    ones_row = nc.dram_tensor("ones_row", [1, P], FP16, kind="ExternalInput")

    out_rows = nc.dram_tensor("out_rows", [NSLICE, C], FP, kind="ExternalOutput")

    with tile.TileContext(nc) as tc:
        # ---- persistent constants -------------------------------------
        with (
            tc.tile_pool(name="const", bufs=1) as const,
            tc.tile_pool(name="wpool", bufs=1) as wpool,
        ):
            mask_s = const.tile([P, 896], FP16)
            ones_c = const.tile([P, 1], FP16)
            ones_r = const.tile([1, P], FP16)
            nc.sync.dma_start(mask_s[:], maskc[:])
            nc.sync.dma_start(ones_c[:], ones_col[:])
            nc.sync.dma_start(ones_r[:], ones_row[:])

            wq_s, wk_s, wv_s = [], [], []
            for cc in range(NCC):
                tq = wpool.tile([P, HPC * D], FP16, tag=f"wq{cc}", name=f"wq{cc}")
                tk = wpool.tile([P, HPC * D], FP16, tag=f"wk{cc}", name=f"wk{cc}")
                tv = wpool.tile([P, HPC * D], FP16, tag=f"wv{cc}", name=f"wv{cc}")
                nc.sync.dma_start(tq[:], wqT[P * cc : P * (cc + 1), :])
                nc.sync.dma_start(tk[:], wkT[P * cc : P * (cc + 1), :])
                nc.sync.dma_start(tv[:], wvT[P * cc : P * (cc + 1), :])
                wq_s.append(tq)
                wk_s.append(tk)
                wv_s.append(tv)

            for rep in range(reps):
                _emit_body(
                    nc, tc, rep, xT, woT, out_rows,
                    cos_t, sin_t, mask_s, ones_c, ones_r, wq_s, wk_s, wv_s,
                    mode=mode,
                )

    moved = _fixup_multiwaits(nc)
    return nc, moved


def _emit_body(nc, tc, rep, xT, woT, out_rows,
               cos_t, sin_t, mask_s, ones_c, ones_r, wq_s, wk_s, wv_s,
               mode="full"):
    HALF = D // 2
    sink = nc.dram_tensor(f"sink_{rep}", [P, 160 * 1024], FP16) if mode.startswith("proj") else None
    # A2A staging, split by head so the first collective can fire while the
    # last batch's h=1 attention is still running, and phase 3 can start
    # accumulating h=0 chunks while the second collective is in flight.
    # ya_in_h rows = 128*shard + d, cols = n within shard.
    ya_in_h = [
        nc.dram_tensor(f"ya_in_{rep}_{h}", [N_CORES * D, NSLICE], FP16)
        for h in range(HPC)
    ]
    ya_out_h = [
        nc.dram_tensor(f"ya_out_{rep}_{h}", [N_CORES * D, NSLICE], FP16)
        for h in range(HPC)
    ]

    with (
        tc.tile_pool(name="xt", bufs=2) as xt_pool,
        tc.tile_pool(name="qkv", bufs=2) as qkv_pool,
        tc.tile_pool(name="ptile", bufs=6) as p_pool,
        tc.tile_pool(name="evict", bufs=3) as e_pool,
        tc.tile_pool(name="small", bufs=2) as s_pool,
        tc.tile_pool(name="ps", bufs=1, space="PSUM") as ps,
    ):
        for b in range(B):
            # ---- QKV projections for batch b --------------------------
            qk_tiles = {}
            for pj in ("q", "k"):
                for h in range(HPC):
                    qk_tiles[(pj, h)] = qkv_pool.tile(
                        [P, T], FP16, tag=f"{pj}T{h}", name=f"{pj}T{h}_{b}"
                    )
            v_tiles = [
                qkv_pool.tile([P, HPC * D], FP16, tag=f"v{tch}", name=f"v{tch}_{b}")
                for tch in range(T // P)
            ]

            TGP = 2 * TG  # 1024-wide projection groups
            for tg in range(T // TGP):
                cos_sl = s_pool.tile([D // 2, TGP], FP, tag="cosS", name=f"cos_{b}_{tg}")
                sin_sl = s_pool.tile([D // 2, TGP], FP, tag="sinS", name=f"sin_{b}_{tg}")
                nc.sync.dma_start(cos_sl[:], cos_t[:, TGP * tg : TGP * (tg + 1)])
                nc.sync.dma_start(sin_sl[:], sin_t[:, TGP * tg : TGP * (tg + 1)])
                xts = []
                for cc in range(NCC):
                    xt = xt_pool.tile([P, TGP], FP16, tag=f"xt{cc}", name=f"xt{cc}_{b}_{tg}")
                    nc.sync.dma_start(
                        xt[:], xT[b, P * cc : P * (cc + 1), TGP * tg : TGP * (tg + 1)]
                    )
                    xts.append(xt)

                # q, k: transposed orientation [d, t] with fused RoPE evict
                for pj, wt in (("q", wq_s), ("k", wk_s)):
                    for h in range(HPC):
                        pmm = ps.tile([P, TGP], FP, tag="big2", bufs=2, name=f"p{pj}{h}_{b}_{tg}")
                        for half in range(2):
                            for cc in range(NCC):
                                nc.tensor.matmul(
                                    pmm[:, TG * half : TG * (half + 1)],
                                    wt[cc][:, D * h : D * (h + 1)],
                                    xts[cc][:, TG * half : TG * (half + 1)],
                                    start=(cc == 0),
                                    stop=(cc == NCC - 1),
                                )
                        dst = qk_tiles[(pj, h)]
                        t1 = s_pool.tile([HALF, TGP], FP, tag="ropeA", name=f"t1_{b}_{tg}")
                        t2 = s_pool.tile([HALF, TGP], FP, tag="ropeB", name=f"t2_{b}_{tg}")
                        x1 = pmm[0:HALF, :]
                        x2 = pmm[HALF:P, :]
                        dcol = dst[:, TGP * tg : TGP * (tg + 1)]
                        nc.vector.tensor_mul(t1[:], x1, cos_sl[:])
                        nc.vector.tensor_mul(t2[:], x2, sin_sl[:])
                        nc.vector.tensor_sub(dcol[0:HALF, :], t1[:], t2[:])
                        nc.vector.tensor_mul(t1[:], x1, sin_sl[:])
                        nc.vector.tensor_mul(t2[:], x2, cos_sl[:])
                        nc.vector.tensor_add(dcol[HALF:P, :], t1[:], t2[:])

                # v: natural orientation [t, d] for both heads
                for t4 in range(TGP // P):
                    tch = (TGP * tg) // P + t4
                    pv = ps.tile([P, HPC * D], FP, tag="misc", bufs=2, name=f"pv_{b}_{tch}")
                    for cc in range(NCC):
                        nc.tensor.matmul(
                            pv[:],
                            xts[cc][:, P * t4 : P * (t4 + 1)],
                            wv_s[cc][:],
                            start=(cc == 0),
                            stop=(cc == NCC - 1),
                        )
                    nc.scalar.copy(v_tiles[tch][:], pv[:])

            # ---- attention for batch b --------------------------------
            if mode.startswith("proj"):
                for idx, ((pj, h), tl) in enumerate(qk_tiles.items()):
                    nc.sync.dma_start(
                        sink[:, (4 * b + idx) * T : (4 * b + idx) * T + T], tl[:]
                    )
                for tch, vt in enumerate(v_tiles):
                    nc.sync.dma_start(
                        sink[:, 33 * T + (16 * b + tch) * HPC * D : 33 * T + (16 * b + tch) * HPC * D + HPC * D],
                        vt[:],
                    )
                continue
            for h in range(HPC):
                qT = qk_tiles[("q", h)]
                kT = qk_tiles[("k", h)]
                for g in range(NTG):
                    n_i = 4 * g + 4  # causal: tk chunks 0 .. 4g+3 (even count)
                    po = ps.tile([P, TG], FP, tag="acc512", bufs=2, name=f"po_{b}_{h}_{g}")
                    pd = ps.tile([1, TG], FP, tag="misc", bufs=2, name=f"pd_{b}_{h}_{g}")
                    for pi in range(n_i // 2):
                        pss = ps.tile([P, 2 * TG], FP, tag="big2", bufs=2, name=f"ps_{b}_{h}_{g}_{pi}")
                        for half in range(2):
                            i = 2 * pi + half
                            nc.tensor.matmul(
                                pss[:, TG * half : TG * (half + 1)],
                                kT[:, P * i : P * (i + 1)],
                                qT[:, TG * g : TG * (g + 1)],
                                start=True,
                                stop=True,
                            )
                        pt = p_pool.tile([P, 2 * TG], FP16, tag="pT", name=f"pt_{b}_{h}_{g}_{pi}")
                        nc.scalar.activation(pt[:], pss[:], EXP, scale=SCALE)
                        offs = []
                        for half in range(2):
                            i = 2 * pi + half
                            pth = pt[:, TG * half : TG * (half + 1)]
                            r = 0
                            if i >= 4 * g and "nomask" not in mode:
                                r = P * i - TG * g
                                nc.vector.tensor_mul(
                                    pth, pth, mask_s[:, 384 - r : 896 - r]
                                )
                            offs.append((i, pth, r))
                        # masked cols < r are zero: skip them. PVs batched
                        # before denoms so the ones stationary loads once/pair.
                        for i, pth, r in offs:
                            nc.tensor.matmul(
                                po[:, r:TG],
                                v_tiles[i][:, D * h : D * (h + 1)],
                                pth[:, r:TG],
                                start=(i == 0),
                                stop=(i == n_i - 1),
                            )
                        for i, pth, r in offs:
                            nc.tensor.matmul(
                                pd[:, r:TG],
                                ones_c[:],
                                pth[:, r:TG],
                                start=(i == 0),
                                stop=(i == n_i - 1),
                            )
                    recip = s_pool.tile([1, TG], FP16, tag="recip", name=f"rc_{b}_{h}_{g}")
                    with nc.allow_low_precision(reason="softmax denom recip; values O(1e3)"):
                        nc.vector.reciprocal(recip[:], pd[:])
                    prb = ps.tile([P, TG], FP, tag="acc512", bufs=2, name=f"prb_{b}_{h}_{g}")
                    nc.tensor.matmul(prb[:], ones_r[:], recip[:], start=True, stop=True)
                    rb = e_pool.tile([P, TG], FP, tag="rb", name=f"rb_{b}_{h}_{g}")
                    nc.scalar.copy(rb[:], prb[:])
                    yt = e_pool.tile([P, TG], FP16, tag="yt", name=f"yt_{b}_{h}_{g}")
                    nc.vector.tensor_mul(yt[:], po[:], rb[:])
                    # stage for A2A: shard j = n // NSLICE, col = n % NSLICE
                    n0 = T * b + TG * g
                    j = n0 // NSLICE
                    col = n0 % NSLICE
                    row = D * j
                    nc.sync.dma_start(
                        ya_in_h[h][row : row + D, col : col + TG], yt[:]
                    )

    # ---- AllToAll: head-sharded -> row-sharded ------------------------
    if mode.startswith(("proj", "attn")):
        return
    for h in range(HPC):
        nc.gpsimd.collective_compute(
            "AllToAll",
            mybir.AluOpType.bypass,
            replica_groups=[list(range(N_CORES))],
            ins=[ya_in_h[h][:]],
            outs=[ya_out_h[h][:]],
        )

    # ---- output projection on this core's row slice -------------------
    with (
        tc.tile_pool(name="ytp", bufs=1) as yt_pool,
        tc.tile_pool(name="wop", bufs=2) as wo_pool,
        tc.tile_pool(name="outp", bufs=4) as out_pool,
        tc.tile_pool(name="ps3", bufs=4, space="PSUM") as ps3,
    ):
        yts = {}
        for h in range(HPC):
            for j in range(N_CORES):
                cc = HPC * j + h
                yt = yt_pool.tile([P, NSLICE], FP16, tag=f"y{cc}", name=f"y{cc}_{rep}")
                nc.sync.dma_start(yt[:], ya_out_h[h][P * j : P * (j + 1), :])
                yts[cc] = yt
        TG3 = TG
        for jg in range(C // TG3):
            wos = []
            for cc in range(NCC):
                wo = wo_pool.tile([P, TG3], FP16, tag=f"wo{cc}", name=f"wo{cc}_{rep}_{jg}")
                nc.sync.dma_start(
                    wo[:], woT[P * cc : P * (cc + 1), TG3 * jg : TG3 * (jg + 1)]
                )
                wos.append(wo)
            cc_order = [HPC * j + h for h in range(HPC) for j in range(N_CORES)]
            for nt in range(NSLICE // P):
                pout = ps3.tile([P, TG3], FP, tag="out", bufs=4, name=f"pout_{rep}_{jg}_{nt}")
                for ci, cc in enumerate(cc_order):
                    nc.tensor.matmul(
                        pout[:],
                        yts[cc][:, P * nt : P * (nt + 1)],
                        wos[cc][:],
                        start=(ci == 0),
                        stop=(ci == NCC - 1),
                    )
                ot = out_pool.tile([P, TG3], FP, tag="ot", name=f"ot_{rep}_{jg}_{nt}")
                nc.scalar.copy(ot[:], pout[:])
                nc.sync.dma_start(
                    out_rows[P * nt : P * (nt + 1), TG3 * jg : TG3 * (jg + 1)], ot[:]
                )


# ---------------------------------------------------------------------------
# Host-side prep + execution
# ---------------------------------------------------------------------------
def _host_inputs(x, wq, wk, wv, wo):
    xT = np.ascontiguousarray(x.transpose(0, 2, 1)).astype(np.float16)
    woT = np.ascontiguousarray(wo.T).astype(np.float16)

    half = D // 2
    freqs = 1.0 / (10000.0 ** (np.arange(half, dtype=np.float32) / half))
    t = np.arange(T, dtype=np.float32)
    ang = freqs[:, None] * t[None, :]  # [half, T]
    cos_t = np.cos(ang).astype(np.float32)
    sin_t = np.sin(ang).astype(np.float32)

    # maskc[p, n] = 1.0 iff p <= n - 384  (sliced per diagonal offset)
    pp = np.arange(P)[:, None]
    nn = np.arange(896)[None, :]
    maskc = (pp <= nn - 384).astype(np.float16)

    ones_col = np.ones((P, 1), dtype=np.float16)
    ones_row = np.ones((1, P), dtype=np.float16)

    common = dict(
        xT=xT, woT=woT, cos_t=cos_t, sin_t=sin_t, maskc=maskc,
        ones_col=ones_col, ones_row=ones_row,
    )
    in_maps = []
    for r in range(N_CORES):
        rows = slice(HPC * D * r, HPC * D * (r + 1))
        in_maps.append(
            dict(
                common,
                wqT=np.ascontiguousarray(wq[rows, :].T).astype(np.float16),
                wkT=np.ascontiguousarray(wk[rows, :].T).astype(np.float16),
                wvT=np.ascontiguousarray(wv[rows, :].T).astype(np.float16),
            )
        )
    return in_maps


_CACHED = {}


def _get_program(reps=1):
    if reps not in _CACHED:
        _CACHED[reps] = build_program(reps)[0]
    return _CACHED[reps]


def kernel(x, wq, wk, wv, wo):
    nc = _get_program(1)
    in_maps = _host_inputs(
        np.asarray(x, dtype=np.float32),
        np.asarray(wq, dtype=np.float32),
        np.asarray(wk, dtype=np.float32),
        np.asarray(wv, dtype=np.float32),
        np.asarray(wo, dtype=np.float32),
    )
    res = run_bass_kernel_spmd(nc, in_maps, list(range(N_CORES)))
    out = np.concatenate([res.results[r]["out_rows"] for r in range(N_CORES)], axis=0)
    return out.reshape(B, T, C)
